# revision 1
# baseline (speedup 1.0000x reference)
"""DiffJPEG (quality=75) Bass kernel for Trainium2, 8-core data-parallel.

Pipeline per image (all linear maps folded into block-diagonal matmuls):
  A:  DCT-rows + RGB->YCbCr fold (+ row-avgpool fold for chroma)   [f32r]
  T1: PE transpose                                                  [f32r]
  B:  DCT-cols (+ col-avgpool fold for chroma)                      [f32r]
  Q:  quantize/dequantize (magic-number RNE round on DVE/POOL,
      -128 Y level-shift folded into the DC magic tile)
  C:  IDCT-rows (+ 2x upsample fold for chroma)                     [f32r]
  T2: PE transpose                                                  [f32r]
  D:  IDCT-cols + YCbCr->RGB fold + 2x upsample fold (chroma),
      accumulated per output channel in PSUM                        [f32r]
  E:  ACT Relu((P + 128)/255) evacuation + POOL min(.,1.0)

Stage B emits the transposed coefficient matrix, so the Y quant tables are
transposed (C_TABLE is symmetric). Chroma ops are paired into [128,2,256]
tiles so each evacuation/quant pass covers 512 columns.
"""
import sys

sys.path.insert(0, "/opt/trn_rl_repo")

import numpy as np

QUALITY = 75
FACTOR = (200.0 - 2.0 * QUALITY) / 100.0  # 0.5
MAGIC = np.float32(1.5 * 2.0**23)  # RNE rounding constant for |x| < 2^22

Y_TABLE = np.array([
    [16, 11, 10, 16, 24, 40, 51, 61],
    [12, 12, 14, 19, 26, 58, 60, 55],
    [14, 13, 16, 24, 40, 57, 69, 56],
    [14, 17, 22, 29, 51, 87, 80, 62],
    [18, 22, 37, 56, 68, 109, 103, 77],
    [24, 35, 55, 64, 81, 104, 113, 92],
    [49, 64, 78, 87, 103, 121, 120, 101],
    [72, 92, 95, 98, 112, 100, 103, 99]], dtype=np.float32)

C_TABLE = np.array([
    [17, 18, 24, 47, 99, 99, 99, 99],
    [18, 21, 26, 66, 99, 99, 99, 99],
    [24, 26, 56, 99, 99, 99, 99, 99],
    [47, 66, 99, 99, 99, 99, 99, 99],
    [99, 99, 99, 99, 99, 99, 99, 99],
    [99, 99, 99, 99, 99, 99, 99, 99],
    [99, 99, 99, 99, 99, 99, 99, 99],
    [99, 99, 99, 99, 99, 99, 99, 99]], dtype=np.float32)

W_FWD = {
    "y": (0.299, 0.587, 0.114),
    "cb": (-0.168736, -0.331264, 0.5),
    "cr": (0.5, -0.418688, -0.081312),
}
W_BWD = {
    "r": {"cr": 1.402},
    "g": {"cb": -0.344136, "cr": -0.714136},
    "b": {"cb": 1.772},
}

N_CORES = 8
IMGS_PER_CORE = 2
H = W = 512
HC = WC = 256  # chroma


def _dct_mat():
    xg = np.arange(8, dtype=np.float64)
    ug = np.arange(8, dtype=np.float64)
    Dm = 0.5 * np.cos((2.0 * xg[None, :] + 1.0) * ug[:, None] * np.pi / 16.0)
    Dm[0, :] *= 1.0 / np.sqrt(2.0)
    return Dm  # float64 [8,8]


def _constants():
    """All constant operands, as float32 numpy arrays (fed as extra inputs)."""
    D8 = _dct_mat()
    BD = np.kron(np.eye(16), D8)  # [128,128] row-DCT for a 128-row chunk
    BDc = np.kron(np.eye(32), D8)  # [256,256]
    Prow = np.zeros((256, 512))
    r = np.arange(256)
    Prow[r, 2 * r] = 0.5
    Prow[r, 2 * r + 1] = 0.5
    Mc = BDc @ Prow  # [256, 512] row-pool + DCT
    P0 = Mc[:128, :128]
    P1 = Mc[:128, 128:256]

    c = {}
    # stage A stationaries (lhsT = M.T so matmul computes M @ X).
    # The reference's img*255 scaling is folded in here.
    for wname, wv in zip("rgb", W_FWD["y"]):
        c[f"a_y_{wname}"] = (wv * 255.0 * BD).T
    c["b_y"] = BD.T
    c["b_c_k0"] = P0.T
    c["b_c_k1"] = P1.T
    c["c_y"] = BD
    c["c_c_k0"] = 2.0 * P0
    c["c_c_k1"] = 2.0 * P1

    # quantization tiles. Stage B emits the TRANSPOSED coefficient matrix
    # (B = BD @ (BD@X).T = C.T), so the Y table pattern is transposed here.
    # C_TABLE is symmetric so chroma needs no transpose.
    qy = (Y_TABLE.T * FACTOR).astype(np.float32)  # exact in f32
    qc = (C_TABLE * FACTOR).astype(np.float32)
    ry = (1.0 / qy).astype(np.float32)
    rc = (1.0 / qc).astype(np.float32)
    c["q_y"] = np.tile(qy, (16, 64)).astype(np.float32)          # [128, 512]
    c["recipq_y"] = np.tile(ry, (16, 64)).astype(np.float32)
    m = np.full((128, 512), MAGIC, dtype=np.float64)
    m[0::8, 0::8] -= 1024.0 * float(ry[0, 0])
    c["magic_y"] = m.astype(np.float32)
    c["q_c"] = np.tile(qc, (16, 64)).astype(np.float32)          # [128, 512]
    c["recipq_c"] = np.tile(rc, (16, 64)).astype(np.float32)
    c["ident"] = np.eye(128)

    return {k: np.ascontiguousarray(v, dtype=np.float32) for k, v in c.items()}


_CONSTS = _constants()
_PROGRAM = None  # cached (nc)
TRACE = False  # set True (e.g. from test.py) to capture an NTFF profile
LAST_RESULT = None  # BassKernelResults of the most recent kernel() call

_F32_KEYS = ("q_y", "recipq_y", "magic_y", "q_c", "recipq_c")


def _round_f32r(x):
    """Round f32 to the 12-explicit-mantissa-bit f32r grid (RNE), matching
    the on-chip cast, so DMA-bitcast loads carry pre-rounded values."""
    u = x.view(np.uint32).astype(np.uint64)
    drop = 11
    half = np.uint64(1 << (drop - 1))
    low = u & np.uint64((1 << drop) - 1)
    u_hi = u >> np.uint64(drop)
    up = (low > half) | ((low == half) & ((u_hi & np.uint64(1)) == 1))
    u2 = (u_hi + up.astype(np.uint64)) << np.uint64(drop)
    return (u2 & np.uint64(0xFFFFFFFF)).astype(np.uint32).view(np.float32)


def _pack_consts():
    """Concat consts along the free dim: f32 pack, early f32r pack (stages
    A/B + ident), late f32r pack (stages C/D)."""
    offs_f, offs_e, offs_l = {}, {}, {}
    fparts, eparts, lparts = [], [], []
    of = oe = ol = 0
    for k, v in _CONSTS.items():
        if k in _F32_KEYS:
            offs_f[k] = (of, v.shape[1]); fparts.append(v); of += v.shape[1]
        elif k.startswith(("a_", "b_")) or k == "ident":
            offs_e[k] = (oe, v.shape[1]); eparts.append(v); oe += v.shape[1]
        else:
            offs_l[k] = (ol, v.shape[1]); lparts.append(v); ol += v.shape[1]
    return (np.concatenate(fparts, axis=1), offs_f,
            np.concatenate(eparts, axis=1), offs_e,
            np.concatenate(lparts, axis=1), offs_l)


_PACK_F, _OFFS_F, _PACK_E, _OFFS_E, _PACK_L, _OFFS_L = _pack_consts()


def _build_program():
    import concourse.bacc as bacc
    import concourse.mybir as mybir
    from concourse.tile import TileContext

    f32 = mybir.dt.float32
    f32r = mybir.dt.float32r
    ACT_COPY = mybir.ActivationFunctionType.Copy
    MULT = mybir.AluOpType.mult
    ADD = mybir.AluOpType.add
    SUB = mybir.AluOpType.subtract

    nc = bacc.Bacc("TRN2", target_bir_lowering=False, debug=False,
                   num_devices=N_CORES)

    x_d = nc.dram_tensor("xc", [IMGS_PER_CORE, 3, H, W], f32,
                         kind="ExternalInput").ap()
    out_d = nc.dram_tensor("outc", [IMGS_PER_CORE, 3, H, W], f32,
                           kind="ExternalOutput").ap()
    cpackf_d = nc.dram_tensor("cpack_f", list(_PACK_F.shape), f32,
                              kind="ExternalInput").ap()
    cpacke_d = nc.dram_tensor("cpack_e", list(_PACK_E.shape), f32,
                              kind="ExternalInput").ap()
    cpackl_d = nc.dram_tensor("cpack_l", list(_PACK_L.shape), f32,
                              kind="ExternalInput").ap()

    with TileContext(nc) as tc:
        with (
            tc.tile_pool(name="const", bufs=1) as cpool,
            tc.tile_pool(name="data", bufs=2) as dpool,
            tc.tile_pool(name="work", bufs=2) as wpool,
            tc.tile_pool(name="psA", bufs=3, space="PSUM") as psA,
            tc.tile_pool(name="psT", bufs=2, space="PSUM") as psT,
            tc.tile_pool(name="psTc", bufs=1, space="PSUM") as psTc,
            tc.tile_pool(name="psB", bufs=2, space="PSUM") as psB,
        ):
            # ---- constants: batched DMAs (early pack first), sliced views ----
            bias_t = cpool.tile([128, 1], f32, name="bias_t")
            nc.vector.memset(bias_t[:], 128.0 / 255.0)
            # PE warmup: dummy matmuls on a memset tile so the HAM clock
            # gate opens while the first DMAs are in flight.
            wu0 = cpool.tile([128, 16], f32, name="wu0")
            nc.vector.memset(wu0[:], 1.0)
            wu = cpool.tile([128, 16], f32r, name="wu")
            nc.vector.tensor_copy(wu[:], wu0[:])
            cpe = cpool.tile(list(_PACK_E.shape), f32r, name="cpe")
            nc.sync.dma_start(cpe[:], cpacke_d.bitcast(f32r))
            cpf = cpool.tile(list(_PACK_F.shape), f32, name="cpf")
            cpl = cpool.tile(list(_PACK_L.shape), f32r, name="cpl")
            cs = {}
            for k, (o, w) in _OFFS_F.items():
                cs[k] = cpf[:, o:o + w]
            for k, (o, w) in _OFFS_E.items():
                cs[k] = cpe[:, o:o + w]
            for k, (o, w) in _OFFS_L.items():
                cs[k] = cpl[:, o:o + w]
            cs["d_y"] = cs["c_y"]
            gen_n = [0]

            def scale_gen(key, src_ap, factor):
                t = cpool.tile([128, 128], f32r, name=f"g_{key}")
                nc.vector.tensor_scalar_mul(t[:], src_ap, float(factor))
                gen_n[0] += 1
                cs[key] = t[:]

            for ch in ("cb", "cr"):
                for wname, wv in zip("rgb", W_FWD[ch]):
                    for kk in range(2):
                        scale_gen(f"a_{ch}_{wname}_k{kk}",
                                  cs[f"b_c_k{kk}"], wv * 255.0)

            def gen_d_consts():
                for och, terms in W_BWD.items():
                    for cch, wv in terms.items():
                        for kk in range(2):
                            scale_gen(f"d_{och}_{cch}_k{kk}",
                                      cs[f"c_c_k{kk}"], wv)
            wp = psA.tile([128, W], f32, name="wp", tag="psA")
            for _ in range(70):
                nc.tensor.matmul(wp[:16, 0:16], wu[:], wu[:], start=True,
                                 stop=True)

            def mm(out_ps, lhsT_ap, rhs_ap, start, stop):
                nc.tensor.matmul(out_ps, lhsT_ap, rhs_ap,
                                 start=start, stop=stop)

            evac_n = [0]

            def evac(dst_ap, src_ap):
                """PSUM->SBUF evacuation, 3:2 ACT:DVE."""
                if evac_n[0] % 3 != 2:
                    nc.scalar.activation(dst_ap, src_ap, ACT_COPY)
                else:
                    nc.vector.tensor_copy(dst_ap, src_ap)
                evac_n[0] += 1

            def tr_block(dst_ps_ap, src_sb_ap):
                nc.tensor.matmul(dst_ps_ap, src_sb_ap, cs["ident"],
                                 is_transpose=True, start=True, stop=True)

            S = [{} for _ in range(IMGS_PER_CORE)]

            def st_load(img):
                xt = dpool.tile([128, 3, 4, W], f32r, name=f"xt_{img}",
                                tag="xt")
                for k in range(4):
                    for ch in range(3):
                        nc.sync.dma_start(
                            xt[:, ch, k, :],
                            x_d[img, ch, 128 * k:128 * (k + 1), :]
                            .bitcast(f32r))
                S[img]["xt"] = xt

            def _a_y_chunk(img, i):
                xt = S[img]["xt"]
                pa = psA.tile([128, W], f32, name=f"A_{img}_y_{i}", tag="psA")
                for ci, wname in enumerate("rgb"):
                    mm(pa[:], cs[f"a_y_{wname}"], xt[:, ci, i, :],
                       start=(ci == 0), stop=(ci == 2))
                sa = wpool.tile([128, W], f32r, name=f"Asb_{img}_y_{i}",
                                tag="Asb_y", bufs=8)
                evac(sa[:], pa[:])
                S[img]["a_sb"].setdefault("y", []).append(sa)

            def _a_c_chunk(img, cn, i):
                xt = S[img]["xt"]
                pa = psA.tile([128, W], f32, name=f"A_{img}_{cn}_{i}",
                              tag="psA")
                first = True
                for ci, wname in enumerate("rgb"):
                    for kk in range(2):
                        mm(pa[:], cs[f"a_{cn}_{wname}_k{kk}"],
                           xt[:, ci, 2 * i + kk, :],
                           start=first, stop=(ci == 2 and kk == 1))
                        first = False
                sa = wpool.tile([128, W], f32r, name=f"Asb_{img}_{cn}_{i}",
                                tag=f"Asb_{cn}", bufs=4)
                evac(sa[:], pa[:])
                S[img]["a_sb"].setdefault(cn, []).append(sa)

            def st_A(img, cns=("y", "cb", "cr")):
                # interleaved by input-DMA arrival: y chunks as each k-chunk
                # lands, chroma chunks as their k-pairs complete
                S[img].setdefault("a_sb", {})
                if "y" in cns:
                    _a_y_chunk(img, 0)
                    _a_y_chunk(img, 1)
                    _a_c_chunk(img, "cr", 0)
                    _a_c_chunk(img, "cb", 0)
                    _a_y_chunk(img, 2)
                    _a_y_chunk(img, 3)
                    _a_c_chunk(img, "cr", 1)
                    _a_c_chunk(img, "cb", 1)

            def st_T1BQ(img, cn):
                a_sb = S[img]["a_sb"]
                cq = S[img].setdefault("cq", {})
                qtile = cs["q_y"] if cn == "y" else cs["q_c"]
                rtile = cs["recipq_y"] if cn == "y" else cs["recipq_c"]
                if cn == "y":
                    t1 = []
                    for j in range(4):
                        pt = psT.tile([128, H], f32r,
                                      name=f"T1_{img}_y_ps{j}", tag="psT")
                        for i in range(4):
                            tr_block(pt[:, 128 * i:128 * (i + 1)],
                                     a_sb["y"][i][:, 128 * j:128 * (j + 1)])
                        st = wpool.tile([128, H], f32r,
                                        name=f"T1_{img}_y_sb{j}",
                                        tag="T1_y", bufs=3)
                        evac(st[:], pt[:])
                        t1.append(st)
                    outs = []
                    for i in range(4):
                        pb = psB.tile([128, H], f32,
                                      name=f"B_{img}_y_{i}", tag="psB")
                        mm(pb[:], cs["b_y"], t1[i][:], True, True)
                        q1 = wpool.tile([128, H], f32,
                                        name=f"q1_{img}_y_{i}", tag="q1")
                        nc.vector.tensor_tensor(q1[:], pb[:], rtile, MULT)
                        q2 = wpool.tile([128, H], f32,
                                        name=f"q2_{img}_y_{i}", tag="q2")
                        nc.gpsimd.tensor_tensor(q2[:], q1[:],
                                                cs["magic_y"], ADD)
                        q3 = wpool.tile([128, H], f32r,
                                        name=f"cq_{img}_y_{i}", tag="cq_y",
                                        bufs=4)
                        nc.vector.scalar_tensor_tensor(
                            q3[:], q2[:], float(MAGIC), qtile, SUB, MULT)
                        outs.append(q3)
                    cq[cn] = outs
                else:
                    t1p = []
                    for jp in range(2):
                        pt = psTc.tile([128, 2, HC], f32r,
                                      name=f"T1_{img}_{cn}_ps{jp}",
                                      tag="psTc")
                        for jj in range(2):
                            j = 2 * jp + jj
                            for i in range(2):
                                tr_block(
                                    pt[:, jj, 128 * i:128 * (i + 1)],
                                    a_sb[cn][i][:, 128 * j:128 * (j + 1)])
                        st = wpool.tile([128, 2, HC], f32r,
                                        name=f"T1_{img}_{cn}_sb{jp}",
                                        tag=f"T1_{cn}", bufs=2)
                        evac(st[:], pt[:])
                        t1p.append(st)
                    t1 = lambda j: t1p[j // 2][:, j % 2, :]
                    pb = psB.tile([128, 2, HC], f32,
                                  name=f"B_{img}_{cn}", tag="psB")
                    for i in range(2):
                        for kk in range(2):
                            mm(pb[:, i, :], cs[f"b_c_k{kk}"],
                               t1(2 * i + kk), kk == 0, kk == 1)
                    q1 = wpool.tile([128, 2, HC], f32,
                                    name=f"q1_{img}_{cn}", tag="q1")
                    nc.vector.tensor_tensor(q1[:], pb[:], rtile, MULT)
                    q2 = wpool.tile([128, 2, HC], f32,
                                    name=f"q2_{img}_{cn}", tag="q2")
                    nc.gpsimd.tensor_scalar_add(q2[:], q1[:], float(MAGIC))
                    q3 = wpool.tile([128, 2, HC], f32r,
                                    name=f"cq_{img}_{cn}", tag="cq_c")
                    nc.vector.scalar_tensor_tensor(
                        q3[:], q2[:], float(MAGIC), qtile, SUB, MULT)
                    cq[cn] = q3

            def st_CT2(img, cn):
                cq = S[img]["cq"]
                t2 = S[img].setdefault("t2", {})
                if cn == "y":
                    csb = []
                    for i in range(4):
                        pc = psA.tile([128, H], f32, name=f"C_{img}_y_{i}",
                                      tag="psA")
                        mm(pc[:], cs["c_y"], cq["y"][i][:], True, True)
                        sc = wpool.tile([128, H], f32r,
                                        name=f"Csb_{img}_y_{i}",
                                        tag="Csb_y", bufs=4)
                        evac(sc[:], pc[:])
                        csb.append(sc)
                    t2y = []
                    for j in range(4):
                        pt = psT.tile([128, H], f32r,
                                      name=f"T2_{img}_y_ps{j}", tag="psT")
                        for i in range(4):
                            tr_block(pt[:, 128 * i:128 * (i + 1)],
                                     csb[i][:, 128 * j:128 * (j + 1)])
                        st = wpool.tile([128, H], f32r,
                                        name=f"T2_{img}_y_sb{j}",
                                        tag="T2_y", bufs=8)
                        evac(st[:], pt[:])
                        t2y.append(st)
                    t2["y"] = lambda i: t2y[i][:]
                else:
                    csbp = []
                    for ip in range(2):
                        pc = psA.tile([128, 2, HC], f32,
                                      name=f"C_{img}_{cn}_{ip}", tag="psA")
                        for ii in range(2):
                            i = 2 * ip + ii
                            mm(pc[:, ii, :], cs[f"c_c_k{i % 2}"],
                               cq[cn][:, i // 2, :], True, True)
                        sc = wpool.tile([128, 2, HC], f32r,
                                        name=f"Csb_{img}_{cn}_{ip}",
                                        tag="Csb_c", bufs=4)
                        evac(sc[:], pc[:])
                        csbp.append(sc)
                    csl = lambda i: csbp[i // 2][:, i % 2, :]
                    t2c = []
                    for j in range(2):
                        pt = psTc.tile([128, H], f32r,
                                      name=f"T2_{img}_{cn}_ps{j}", tag="psTc")
                        for i in range(4):
                            tr_block(pt[:, 128 * i:128 * (i + 1)],
                                     csl(i)[:, 128 * j:128 * (j + 1)])
                        st = wpool.tile([128, H], f32r,
                                        name=f"T2_{img}_{cn}_sb{j}",
                                        tag=f"T2_{cn}", bufs=4)
                        evac(st[:], pt[:])
                        t2c.append(st)
                    t2[cn] = (lambda tlist: (lambda i: tlist[i][:]))(t2c)

            def st_D(img, och_list=("r", "g", "b")):
                t2 = S[img]["t2"]
                for och in och_list:
                    oi = "rgb".index(och)
                    ot = dpool.tile([128, 4, W], f32, name=f"ot_{img}_{och}",
                                    tag="ot")
                    for i in range(4):
                        pd = psB.tile([128, W], f32, name=f"D_{img}_{och}_{i}",
                                      tag="psB")
                        mm(pd[:], cs["d_y"], t2["y"](i), True, False)
                        terms = list(W_BWD[och].items())
                        for ti, (cch, _) in enumerate(terms):
                            mm(pd[:], cs[f"d_{och}_{cch}_k{i % 2}"],
                               t2[cch](i // 2),
                               False, ti == len(terms) - 1)
                        if i in (1, 2):
                            # tail chunks: DVE 2-op path keeps ACT off the
                            # final critical chain
                            fe = wpool.tile([128, W], f32,
                                            name=f"fin_{img}_{och}_{i}",
                                            tag="fin")
                            nc.vector.tensor_scalar(fe[:], pd[:], 128.0,
                                                    1.0 / 255.0, ADD, MULT)
                            nc.vector.tensor_scalar(ot[:, i, :], fe[:], 1.0,
                                                    0.0,
                                                    mybir.AluOpType.min,
                                                    mybir.AluOpType.max)
                        else:
                            fe = wpool.tile([128, W], f32,
                                            name=f"fin_{img}_{och}_{i}",
                                            tag="fin")
                            nc.scalar.activation(fe[:], pd[:],
                                                 mybir.ActivationFunctionType.Relu,
                                                 bias=bias_t[:],
                                                 scale=1.0 / 255.0)
                            if i % 2 == 0:
                                nc.gpsimd.tensor_scalar_min(ot[:, i, :],
                                                            fe[:], 1.0)
                            else:
                                nc.vector.tensor_scalar_min(ot[:, i, :],
                                                            fe[:], 1.0)
                        if i % 2 == 1:
                            hh = i // 2
                            nc.sync.dma_start(
                                out_d[img, oi, 256 * hh:256 * (hh + 1), :]
                                .rearrange("(k p) w -> p k w", p=128),
                                ot[:, 2 * hh:2 * hh + 2, :])

            # interleaved emission: keep both images in flight
            st_load(0)
            st_A(0)
            st_load(1)
            nc.sync.dma_start(cpf[:], cpackf_d)
            nc.sync.dma_start(cpl[:], cpackl_d.bitcast(f32r))
            gen_d_consts()
            st_A(1)
            for cn in ("y", "cb", "cr"):
                st_T1BQ(0, cn)
                st_T1BQ(1, cn)
            st_CT2(0, "y")
            st_CT2(0, "cb")
            st_CT2(1, "y")
            st_CT2(0, "cr")
            st_D(0, ("r",))
            st_CT2(1, "cb")
            st_D(0, ("g",))
            st_CT2(1, "cr")
            st_D(0, ("b",))
            st_D(1, ("r", "g"))
            st_D(1, ("b",))

    nc.compile()
    return nc


def kernel(x: np.ndarray) -> np.ndarray:
    global _PROGRAM, LAST_RESULT
    from concourse.bass_utils import run_bass_kernel_spmd

    x = np.ascontiguousarray(np.asarray(x, dtype=np.float32))
    assert x.shape == (N_CORES * IMGS_PER_CORE, 3, H, W)

    if _PROGRAM is None:
        _PROGRAM = _build_program()
    nc = _PROGRAM

    in_maps = []
    for c in range(N_CORES):
        m = {"xc": x[IMGS_PER_CORE * c:IMGS_PER_CORE * (c + 1)],
             "cpack_f": _PACK_F, "cpack_e": _PACK_E, "cpack_l": _PACK_L}
        in_maps.append(m)

    res = run_bass_kernel_spmd(nc, in_maps, list(range(N_CORES)), trace=TRACE)
    LAST_RESULT = res
    out = np.concatenate([res.results[c]["outc"] for c in range(N_CORES)],
                         axis=0)
    return out



# revision 5
# speedup vs baseline: 1.3064x; 1.3064x over previous
"""DiffJPEG (quality=75) Bass kernel for Trainium2, 8-core data-parallel.

v2 pipeline per image — zero PE transposes, both transpose stages fused
into neighboring matmuls via the stationary operand:
  conv:  x f32 -> fp16 tiles (input precision: fp16, validated rel<0.01)
  A+T1:  t1 = (rowDCT+color @ X).T computed directly as
         X_block.T @ (w*255*BD).T per output block (fp16 matmuls),
         Y level shift (-362.039) folded into the t1 evac DC columns.
  B:     col-DCT (+col-pool for chroma), f32r 512-wide matmuls.
  Q:     q1 = P*recip (DVE), q2 = +MAGIC (ACT copy w/ float bias),
         q3 = (q2-MAGIC)*q -> fp16 (DVE stt); tables are [128,8] tiles
         broadcast along the free dim via stride-0 APs.
  C+T2:  t2 = cq.T @ IDCT-consts per block (fp16), +128 output level
         folded into the Y t2-evac bias (per-partition, ACT Identity).
  D:     col-IDCT + color + upsample folds, consts pre-scaled 1/255 so
         PSUM holds final pixels in [0,1]; fp16 matmuls, 512-wide.
  fin:   single (min 1, max 0) tensor_scalar per chunk, then DMA out.
"""
import sys

sys.path.insert(0, "/opt/trn_rl_repo")

import numpy as np

QUALITY = 75
FACTOR = (200.0 - 2.0 * QUALITY) / 100.0  # 0.5
MAGIC = np.float32(1.5 * 2.0 ** 23)
LS = np.float64(128.0 * 8.0 * 0.5 / np.sqrt(2.0))  # 362.0386719675...

Y_TABLE = np.array([
    [16, 11, 10, 16, 24, 40, 51, 61],
    [12, 12, 14, 19, 26, 58, 60, 55],
    [14, 13, 16, 24, 40, 57, 69, 56],
    [14, 17, 22, 29, 51, 87, 80, 62],
    [18, 22, 37, 56, 68, 109, 103, 77],
    [24, 35, 55, 64, 81, 104, 113, 92],
    [49, 64, 78, 87, 103, 121, 120, 101],
    [72, 92, 95, 98, 112, 100, 103, 99]], dtype=np.float64)

C_TABLE = np.array([
    [17, 18, 24, 47, 99, 99, 99, 99],
    [18, 21, 26, 66, 99, 99, 99, 99],
    [24, 26, 56, 99, 99, 99, 99, 99],
    [47, 66, 99, 99, 99, 99, 99, 99],
    [99, 99, 99, 99, 99, 99, 99, 99],
    [99, 99, 99, 99, 99, 99, 99, 99],
    [99, 99, 99, 99, 99, 99, 99, 99],
    [99, 99, 99, 99, 99, 99, 99, 99]], dtype=np.float64)

W_FWD = {
    "y": (0.299, 0.587, 0.114),
    "cb": (-0.168736, -0.331264, 0.5),
    "cr": (0.5, -0.418688, -0.081312),
}
W_BWD = {
    "r": {"cr": 1.402},
    "g": {"cb": -0.344136, "cr": -0.714136},
    "b": {"cb": 1.772},
}

N_CORES = 8
IMGS_PER_CORE = 2
H = W = 512


def _round_f32r(x):
    """Round f32 to the 12-explicit-mantissa-bit f32r grid (RNE)."""
    x = np.ascontiguousarray(x, dtype=np.float32)
    u = x.view(np.uint32).astype(np.uint64)
    drop = 11
    half = np.uint64(1 << (drop - 1))
    low = u & np.uint64((1 << drop) - 1)
    u_hi = u >> np.uint64(drop)
    up = (low > half) | ((low == half) & ((u_hi & np.uint64(1)) == 1))
    u2 = (u_hi + up.astype(np.uint64)) << np.uint64(drop)
    return (u2 & np.uint64(0xFFFFFFFF)).astype(np.uint32).view(np.float32)


def _dct_mat():
    xg = np.arange(8, dtype=np.float64)
    ug = np.arange(8, dtype=np.float64)
    Dm = 0.5 * np.cos((2.0 * xg[None, :] + 1.0) * ug[:, None] * np.pi / 16.0)
    Dm[0, :] *= 1.0 / np.sqrt(2.0)
    return Dm


def _constants():
    D8 = _dct_mat()
    BD128 = np.kron(np.eye(16), D8)  # [128,128]
    P = np.zeros((128, 256))
    idx = np.arange(128)
    P[idx, 2 * idx] = 0.5
    P[idx, 2 * idx + 1] = 0.5
    M = np.kron(np.eye(16), D8) @ P  # [128, 256] row-pool + DCT
    P0, P1 = M[:, :128], M[:, 128:]

    # f32r pack: B-stage stationaries
    b_y = _round_f32r(BD128.T)
    b_c_k0 = _round_f32r(P0.T)
    b_c_k1 = _round_f32r(P1.T)
    pack_r = np.concatenate([b_y, b_c_k0, b_c_k1], axis=1)  # [128, 384]

    # fp16 pack: CT2 moving consts
    bd = np.asarray(BD128, dtype=np.float16)
    cc0 = np.asarray(2.0 * P0, dtype=np.float16)
    cc1 = np.asarray(2.0 * P1, dtype=np.float16)
    pack_h = np.concatenate([bd, cc0, cc1], axis=1)  # [128, 384] fp16

    # f32 pack: quant tables [128,8] x4 + bias_y [128,1]
    qy = np.tile((Y_TABLE.T * FACTOR), (16, 1)).astype(np.float32)
    qc = np.tile((C_TABLE.T * FACTOR), (16, 1)).astype(np.float32)
    ry = (1.0 / qy).astype(np.float32)
    rc = (1.0 / qc).astype(np.float32)
    bias_y = np.zeros((128, 1), dtype=np.float32)
    bias_y[0::8, 0] = np.float32(LS)
    pack_f = np.concatenate([qy, ry, qc, rc, bias_y], axis=1)  # [128, 33]

    return (np.ascontiguousarray(pack_r, dtype=np.float32),
            np.ascontiguousarray(pack_h, dtype=np.float16),
            np.ascontiguousarray(pack_f, dtype=np.float32))


_PACK_R, _PACK_H, _PACK_F = _constants()
_PROGRAM = None
TRACE = False
LAST_RESULT = None


def _build_program():
    import concourse.bacc as bacc
    import concourse.mybir as mybir
    from concourse.tile import TileContext

    f32 = mybir.dt.float32
    f32r = mybir.dt.float32r
    f16 = mybir.dt.float16
    ACT_COPY = mybir.ActivationFunctionType.Copy
    ACT_IDENT = mybir.ActivationFunctionType.Identity
    ADD = mybir.AluOpType.add
    SUB = mybir.AluOpType.subtract
    MULT = mybir.AluOpType.mult
    MIN = mybir.AluOpType.min
    MAX = mybir.AluOpType.max

    nc = bacc.Bacc("TRN2", target_bir_lowering=False, debug=False,
                   num_devices=N_CORES)

    x_d = nc.dram_tensor("xc", [IMGS_PER_CORE, 3, H, W], f32,
                         kind="ExternalInput").ap()
    out_d = nc.dram_tensor("outc", [IMGS_PER_CORE, 3, H, W], f32,
                           kind="ExternalOutput").ap()
    packr_d = nc.dram_tensor("pack_r", list(_PACK_R.shape), f32,
                             kind="ExternalInput").ap()
    packh_d = nc.dram_tensor("pack_h", list(_PACK_H.shape), f16,
                             kind="ExternalInput").ap()
    packf_d = nc.dram_tensor("pack_f", list(_PACK_F.shape), f32,
                             kind="ExternalInput").ap()

    with TileContext(nc) as tc:
        with (
            tc.tile_pool(name="const", bufs=1) as cpool,
            tc.tile_pool(name="data", bufs=2) as dpool,
            tc.tile_pool(name="work", bufs=2) as wpool,
            tc.tile_pool(name="psA", bufs=2, space="PSUM") as psA,
            tc.tile_pool(name="psB", bufs=2, space="PSUM") as psB,
            tc.tile_pool(name="psT", bufs=2, space="PSUM") as psT,
            tc.tile_pool(name="psD", bufs=2, space="PSUM") as psD,
        ):
            # ---- PE warmup: dummy matmuls while DMAs are in flight ----
            wu0 = cpool.tile([128, 16], f32, name="wu0")
            nc.vector.memset(wu0[:], 1.0)
            wu = cpool.tile([128, 16], f32r, name="wu")
            nc.vector.tensor_copy(wu[:], wu0[:])

            # ---- constant DMAs ----
            cr_t = cpool.tile([128, 384], f32r, name="cr_t")
            nc.sync.dma_start(cr_t[:], packr_d.bitcast(f32r))
            ch_t = cpool.tile([128, 384], f16, name="ch_t")
            nc.sync.dma_start(ch_t[:], packh_d)
            cf_t = cpool.tile([128, 33], f32, name="cf_t")
            nc.sync.dma_start(cf_t[:], packf_d)

            cs = {
                "b_y": cr_t[:, 0:128],
                "b_c_k0": cr_t[:, 128:256],
                "b_c_k1": cr_t[:, 256:384],
                "bd": ch_t[:, 0:128],
                "cc0": ch_t[:, 128:256],
                "cc1": ch_t[:, 256:384],
                "qy": cf_t[:, 0:8],
                "ry": cf_t[:, 8:16],
                "qc": cf_t[:, 16:24],
                "rc": cf_t[:, 24:32],
                "bias_y": cf_t[:, 32:33],
            }

            wp = psA.tile([128, W], f32, name="wp", tag="psA")
            for _ in range(110):
                nc.tensor.matmul(wp[:16, 0:16], wu[:], wu[:], start=True,
                                 stop=True)

            # ---- on-chip generated fp16 consts ----
            def gen16(key, src_ap, factor, width=128):
                t = cpool.tile([128, width], f16, name=f"g_{key}")
                nc.vector.tensor_scalar_mul(t[:], src_ap, float(factor))
                cs[key] = t[:]

            for wname, wv in zip("rgb", W_FWD["y"]):
                gen16(f"ay_{wname}", cs["b_y"], wv * 255.0)
            for cn in ("cb", "cr"):
                for wname, wv in zip("rgb", W_FWD[cn]):
                    gen16(f"a_{cn}_{wname}_k0", cs["b_c_k0"][:, 0:64],
                          wv * 255.0, width=64)
                    gen16(f"a_{cn}_{wname}_k1", cs["b_c_k1"][:, 64:128],
                          wv * 255.0, width=64)
            gen16("dd_y", cs["bd"], 1.0 / 255.0)
            for och, terms in W_BWD.items():
                for cch, wv in terms.items():
                    for k in (0, 1):
                        gen16(f"d_{och}_{cch}_k{k}", cs[f"cc{k}"],
                              wv / 255.0)

            def bc8(key, reps):
                """broadcast a [128,8] table along new dim: [128,*reps,8]."""
                ap = cs[key]
                for _ in range(len(reps)):
                    ap = ap.unsqueeze(1)
                return ap.broadcast_to([128, *reps, 8])

            def mm(out_ps, lhsT_ap, rhs_ap, start, stop):
                nc.tensor.matmul(out_ps, lhsT_ap, rhs_ap,
                                 start=start, stop=stop)

            S = [{} for _ in range(IMGS_PER_CORE)]

            # ---------------- stages ----------------
            def st_load(img):
                xt = dpool.tile([128, 3, 4, W], f32, name=f"xt_{img}",
                                tag="xt")
                for ch in range(3):
                    for k in range(4):
                        nc.sync.dma_start(
                            xt[:, ch, k, :],
                            x_d[img, ch, 128 * k:128 * (k + 1), :])
                S[img]["xt"] = xt

            CONV_ENG = ("act", "pool", "dve", "pool", "act", "pool",
                        "dve", "pool", "act", "pool", "act", "pool")

            def st_conv(img, chans=(0, 1, 2)):
                xt = S[img]["xt"]
                x16 = S[img].get("x16")
                if x16 is None:
                    x16 = dpool.tile([128, 3, 4, W], f16, name=f"x16_{img}",
                                     tag="x16")
                    S[img]["x16"] = x16
                for ch in chans:
                    for k in range(4):
                        eng = CONV_ENG[(ch * 4 + k) % len(CONV_ENG)]
                        if eng == "act":
                            nc.scalar.activation(x16[:, ch, k, :],
                                                 xt[:, ch, k, :], ACT_COPY)
                        elif eng == "dve":
                            nc.vector.tensor_copy(x16[:, ch, k, :],
                                                  xt[:, ch, k, :])
                        else:
                            nc.gpsimd.tensor_copy(x16[:, ch, k, :],
                                                  xt[:, ch, k, :])

            def st_AT1y(img, j):
                """fused A+T1 for Y, output column-chunk j -> t1y[j]."""
                x16 = S[img]["x16"]
                pa = psA.tile([128, W], f32, name=f"AT1y_{img}_{j}",
                              tag="psA")
                for i in range(4):
                    for ci, wname in enumerate("rgb"):
                        mm(pa[:, 128 * i:128 * (i + 1)],
                           x16[:, ci, i, 128 * j:128 * (j + 1)],
                           cs[f"ay_{wname}"], ci == 0, ci == 2)
                t1 = wpool.tile([128, W], f32r, name=f"t1y_{img}_{j}",
                                tag="t1y", bufs=8)
                pav = pa[:].rearrange("p (a b) -> p a b", b=8)
                t1v = t1[:].rearrange("p (a b) -> p a b", b=8)
                # DC columns get the Y level shift; rest plain copy
                nc.vector.tensor_scalar_add(t1v[:, :, 0], pav[:, :, 0],
                                            -float(LS))
                nc.vector.tensor_copy(t1v[:, :, 1:8], pav[:, :, 1:8])
                S[img].setdefault("t1y", {})[j] = t1

            def st_AT1c(img, cn, jp):
                """fused A+T1 chroma: j-pair jp -> t1c[(cn, jp)]
                [128, 2, 256]."""
                x16 = S[img]["x16"]
                pa = psA.tile([128, W], f32, name=f"AT1c_{img}_{cn}_{jp}",
                              tag="psA")
                pav = pa[:].rearrange("p (a b) -> p a b", b=256)
                for jj in range(2):
                    j = 2 * jp + jj
                    for i in range(2):
                        for k in range(2):
                            for ci, wname in enumerate("rgb"):
                                mm(pav[:, jj,
                                       128 * i + 64 * k:128 * i + 64 * (k + 1)],
                                   x16[:, ci, 2 * i + k,
                                       128 * j:128 * (j + 1)],
                                   cs[f"a_{cn}_{wname}_k{k}"],
                                   ci == 0, ci == 2)
                t1 = wpool.tile([128, 2, 256], f32r,
                                name=f"t1c_{img}_{cn}_{jp}", tag="t1c",
                                bufs=8)
                nc.scalar.activation(t1[:], pav[:], ACT_COPY)
                S[img].setdefault("t1c", {})[(cn, jp)] = t1

            def st_BQy(img, i):
                t1 = S[img]["t1y"][i]
                pb = psB.tile([128, W], f32, name=f"B_{img}_y_{i}",
                              tag="psB")
                mm(pb[:], cs["b_y"], t1[:], True, True)
                pbv = pb[:].rearrange("p (a b) -> p a b", b=8)
                w1 = wpool.tile([128, W], f32, name=f"q1_{img}_y_{i}",
                                tag="q1", bufs=3)
                w1v = w1[:].rearrange("p (a b) -> p a b", b=8)
                nc.vector.tensor_tensor(w1v, pbv, bc8("ry", (64,)), MULT)
                w2 = wpool.tile([128, W], f32, name=f"q2_{img}_y_{i}",
                                tag="q2", bufs=3)
                nc.gpsimd.tensor_scalar(w2[:], w1[:], float(MAGIC),
                                        float(MAGIC), ADD, SUB)
                cq = wpool.tile([128, W], f16, name=f"cq_{img}_y_{i}",
                                tag="cqy", bufs=8)
                cqv = cq[:].rearrange("p (a b) -> p a b", b=8)
                w2v = w2[:].rearrange("p (a b) -> p a b", b=8)
                nc.vector.tensor_tensor(cqv, w2v, bc8("qy", (64,)), MULT)
                S[img].setdefault("cqy", {})[i] = cq

            def st_BQc(img, cn):
                t1c = S[img]["t1c"]
                pb0 = psB.tile([128, W], f32, name=f"B_{img}_{cn}",
                               tag="psB")
                pb = pb0[:].rearrange("p (c a) -> p c a", c=2)
                for b in range(2):
                    for k in range(2):
                        mm(pb[:, b, :], cs[f"b_c_k{k}"],
                           t1c[(cn, b)][:, k, :], k == 0, k == 1)
                pbv = pb0[:].rearrange("p (a b) -> p a b", b=8)
                w1 = wpool.tile([128, W], f32, name=f"q1_{img}_{cn}",
                                tag="q1c", bufs=2)
                w1v = w1[:].rearrange("p (a b) -> p a b", b=8)
                nc.vector.tensor_tensor(w1v, pbv, bc8("rc", (64,)), MULT)
                w2 = wpool.tile([128, W], f32, name=f"q2_{img}_{cn}",
                                tag="q2c", bufs=2)
                nc.gpsimd.tensor_scalar(w2[:], w1[:], float(MAGIC),
                                        float(MAGIC), ADD, SUB)
                cq = wpool.tile([128, 2, 256], f16, name=f"cq_{img}_{cn}",
                                tag="cqc", bufs=4)
                cqv = cq[:].rearrange("p c u -> p (c u)")\
                    .rearrange("p (a b) -> p a b", b=8)
                w2v = w2[:].rearrange("p (a b) -> p a b", b=8)
                nc.vector.tensor_tensor(cqv, w2v, bc8("qc", (64,)), MULT)
                S[img].setdefault("cqc", {})[cn] = cq

            def st_CT2y(img, j):
                cqy = S[img]["cqy"]
                pt = psT.tile([128, W], f32, name=f"CT2y_{img}_{j}",
                              tag="psT")
                for i in range(4):
                    mm(pt[:, 128 * i:128 * (i + 1)],
                       cqy[i][:, 128 * j:128 * (j + 1)], cs["bd"],
                       True, True)
                t2 = wpool.tile([128, W], f16, name=f"t2y_{img}_{j}",
                                tag="t2y", bufs=8)
                nc.scalar.activation(t2[:], pt[:], ACT_IDENT,
                                     bias=cs["bias_y"], scale=1.0)
                S[img].setdefault("t2y", {})[j] = t2

            def st_CT2c(img, cn, j):
                cq = S[img]["cqc"][cn]
                pt = psT.tile([128, W], f32, name=f"CT2c_{img}_{cn}_{j}",
                              tag="psT")
                for i in range(4):
                    mm(pt[:, 128 * i:128 * (i + 1)],
                       cq[:, i // 2, 128 * j:128 * (j + 1)],
                       cs[f"cc{i % 2}"], True, True)
                t2 = wpool.tile([128, W], f16, name=f"t2c_{img}_{cn}_{j}",
                                tag="t2c", bufs=8)
                nc.scalar.activation(t2[:], pt[:], ACT_COPY)
                S[img].setdefault("t2c", {})[(cn, j)] = t2

            def st_D(img, och_list=("r", "g", "b")):
                t2y = S[img]["t2y"]
                t2c = S[img]["t2c"]
                for och in och_list:
                    oi = "rgb".index(och)
                    ot = dpool.tile([128, 4, W], f32, name=f"ot_{img}_{och}",
                                    tag="ot")
                    for i in range(4):
                        pd = psD.tile([128, W], f32,
                                      name=f"D_{img}_{och}_{i}", tag="psD")
                        terms = list(W_BWD[och].items())
                        mm(pd[:], cs["dd_y"], t2y[i][:], True, False)
                        for ti, (cch, _) in enumerate(terms):
                            mm(pd[:], cs[f"d_{och}_{cch}_k{i % 2}"],
                               t2c[(cch, i // 2)][:],
                               False, ti == len(terms) - 1)
                        if i % 2 == 0:
                            fe = wpool.tile([128, W], f32,
                                            name=f"fin_{img}_{och}_{i}",
                                            tag="fin", bufs=2)
                            nc.scalar.activation(
                                fe[:], pd[:],
                                mybir.ActivationFunctionType.Relu)
                            nc.gpsimd.tensor_scalar_min(ot[:, i, :],
                                                        fe[:], 1.0)
                        else:
                            nc.vector.tensor_scalar(ot[:, i, :], pd[:],
                                                    1.0, 0.0, MIN, MAX)
                        if i % 2 == 1:
                            hh = i // 2
                            nc.sync.dma_start(
                                out_d[img, oi, 256 * hh:256 * (hh + 1), :]
                                .rearrange("(k p) w -> p k w", p=128),
                                ot[:, 2 * hh:2 * hh + 2, :])

            # ---------------- emission schedule ----------------
            st_load(0)
            st_conv(0)
            for j in range(4):
                st_AT1y(0, j)
            st_load(1)
            for cn in ("cb", "cr"):
                for jp in range(2):
                    st_AT1c(0, cn, jp)
            st_conv(1)
            for i in range(4):
                st_BQy(0, i)
            st_BQc(0, "cb")
            st_BQc(0, "cr")
            for j in range(4):
                st_AT1y(1, j)
            for j in range(4):
                st_CT2y(0, j)
            for cn in ("cb", "cr"):
                for jp in range(2):
                    st_AT1c(1, cn, jp)
            for cn in ("cb", "cr"):
                for j in range(2):
                    st_CT2c(0, cn, j)
            st_D(0, ("r",))
            for i in range(4):
                st_BQy(1, i)
            st_D(0, ("g",))
            st_BQc(1, "cb")
            st_BQc(1, "cr")
            st_D(0, ("b",))
            for j in range(4):
                st_CT2y(1, j)
            for cn in ("cb", "cr"):
                for j in range(2):
                    st_CT2c(1, cn, j)
            st_D(1, ("r", "g", "b"))

    nc.compile()
    return nc


def kernel(x: np.ndarray) -> np.ndarray:
    global _PROGRAM, LAST_RESULT
    from concourse.bass_utils import run_bass_kernel_spmd

    x = np.ascontiguousarray(np.asarray(x, dtype=np.float32))
    assert x.shape == (N_CORES * IMGS_PER_CORE, 3, H, W)

    if _PROGRAM is None:
        _PROGRAM = _build_program()
    nc = _PROGRAM

    in_maps = []
    for c in range(N_CORES):
        m = {"xc": x[IMGS_PER_CORE * c:IMGS_PER_CORE * (c + 1)],
             "pack_r": _PACK_R, "pack_h": _PACK_H, "pack_f": _PACK_F}
        in_maps.append(m)

    res = run_bass_kernel_spmd(nc, in_maps, list(range(N_CORES)), trace=TRACE)
    LAST_RESULT = res
    out = np.concatenate([res.results[c]["outc"] for c in range(N_CORES)],
                         axis=0)
    return out
